# revision 28
# baseline (speedup 1.0000x reference)
"""DiffGraphTransformer attention kernel for 8x Trainium2 NeuronCores.

Reference computation (T=1024, B=8, E=512, H=8, hd=64):
    qkv = query @ in_proj_weight.T + in_proj_bias ; q,k,v = split(qkv)
    k = q ; q *= hd**-0.5
    per (batch,head): scores = q @ k.T            (T,T)
                      w = exp(scores - max) * pe[b]
                      w /= clip(sum(w,-1), 1e-6)
                      attn = w @ v
    out = attn @ out_proj_weight.T + out_proj_bias
Sharding: batch b -> core b (pure SPMD, no collectives).

Design notes (v1.2):
  * k == q: the k-chunk of in_proj is dead weight.  Softmax max-subtraction
    replaced by a constant shift exp(s/8 - 10) that cancels in the
    normalization.  E = exp(q q^T) is symmetric, so storing it [s, t] and
    multiplying by pe^T yields the contraction-major attention operand with
    no transpose.  attention lhsT = [v_h | ones]: PSUM row 64 = denominator.
  * ACT (exp) is the floor: 64 x (128,1024) exps ~ 1.2-1.3us each.  The
    whole operand path is fp16 (same PE speed, half the DMA, 2x DVE).
  * Input DMAs are batched ONE per tensor (a dma_start costs ~730ns of
    serial issue time on the sync queue; 30 separate loads = 22us of ramp).
  * Normalization is DMA-free: denominator rows are copied to a (2,512)
    tile per nh, reciprocal'd in one cheap approx DVE op (51-ULP is far
    below the error budget; denominators are O(0.1..100) so no edge cases),
    partition-broadcast with a K=2 selector MATMUL on the PE, and applied
    fused with the PSUM evacuation (attnT = psum * rm).  No DRAM bounce,
    no sync-queue traffic, ~3us chain latency instead of ~13us.
  * PSUM: scores 2x(128,1024) [4 banks] + attention accumulators 4x(65,512)
    [4 banks].  The broadcast matmuls borrow the scores ring during the
    body (pair 3's chain borrows ps_b; its accumulators split ps_a/ps_b).
  * Biases are applied even though setup_inputs zeroes them (bq in the qT
    evacuation, v-bias folded into bo2, added via a K=1 ones matmul).
"""

import sys

for _p in ("/opt/trn_rl_repo",):
    if _p not in sys.path:
        sys.path.insert(0, _p)

import numpy as np

T, B, E = 1024, 8, 512
H = 8
HD = E // H  # 64
N_CORES = 8

EXP_SHIFT = -10.0

_cache = {}


def _build_nc():
    import concourse.bass as bass
    import concourse.tile as tile
    import concourse.mybir as mybir
    from concourse import bacc
    from contextlib import ExitStack

    f32 = mybir.dt.float32
    fp16 = mybir.dt.float16
    Exp = mybir.ActivationFunctionType.Exp

    nc = bacc.Bacc("TRN2", debug=False)

    KT = E // 128   # 4 contraction tiles for the projections
    TT = T // 128   # 8 t-tiles
    NH = T // 512   # 2 psum-bank halves of the t dimension
    NP = H // 2     # head pairs

    # DRAM I/O (per-core contents supplied via in_maps).  Folded layouts so
    # each tensor loads in ONE dma_start.
    xT_d = nc.dram_tensor("xT", [E, T], fp16, kind="ExternalInput").ap()
    peT_d = nc.dram_tensor("peT", [T, T], fp16, kind="ExternalInput").ap()
    wqT_d = nc.dram_tensor("wqT", [E, E], fp16, kind="ExternalInput").ap()
    wvT_d = nc.dram_tensor("wvT", [E, E], fp16, kind="ExternalInput").ap()
    woT_d = nc.dram_tensor("woT", [E, E], fp16, kind="ExternalInput").ap()
    bq_d = nc.dram_tensor("bq", [E], f32, kind="ExternalInput").ap()
    bo2_d = nc.dram_tensor("bo2", [E], fp16, kind="ExternalInput").ap()
    out_d = nc.dram_tensor("out", [T, E], f32, kind="ExternalOutput").ap()

    with ExitStack() as ctx:
        tc = ctx.enter_context(tile.TileContext(nc))

        sing = ctx.enter_context(tc.tile_pool(name="sing", bufs=1))
        p_in = ctx.enter_context(tc.tile_pool(name="p_in", bufs=1))
        p_qv = ctx.enter_context(tc.tile_pool(name="p_qv", bufs=1))
        p_E = ctx.enter_context(tc.tile_pool(name="p_E", bufs=12))
        p_W = ctx.enter_context(tc.tile_pool(name="p_W", bufs=20))
        p_rr = ctx.enter_context(tc.tile_pool(name="p_rr", bufs=2))
        p_rm = ctx.enter_context(tc.tile_pool(name="p_rm", bufs=2))
        p_st = ctx.enter_context(tc.tile_pool(name="p_st", bufs=2))
        ps_a = ctx.enter_context(tc.tile_pool(name="ps_a", bufs=2, space="PSUM"))
        ps_b = ctx.enter_context(tc.tile_pool(name="ps_b", bufs=4, space="PSUM"))

        # ---- t=0: ACT table preload + PE warm-up spam ---------------------
        dmy = sing.tile([1, 16], f32, tag="dmy")
        nc.vector.memset(dmy, 0.0)
        dmy2 = sing.tile([1, 16], f32, tag="dmy2")
        nc.scalar.activation(out=dmy2, in_=dmy, func=Exp, scale=1.0, bias=0.0)
        warm = sing.tile([128, 512], fp16, tag="warm")
        nc.vector.memset(warm, 0.0)
        for r in range(10):
            wps = ps_b.tile([128, 512], f32, tag="slot", name="wps")
            nc.tensor.matmul(wps, warm[:, 0:128], warm, start=True, stop=True)

        # selectors for the K=1 broadcast matmuls (data on partition 64 to
        # match the PSUM denominator rows): sel_h[64, m] = 1 iff head-half h
        # owns output row m
        sel_h = []
        for hh in range(2):
            s = sing.tile([HD + 1, 128], f32, tag=f"sel{hh}", name="sel")
            nc.vector.memset(s, 0.0)
            nc.vector.memset(s[HD:HD + 1, hh * HD:(hh + 1) * HD], 1.0)
            sel_h.append(s)
        ones1 = sing.tile([1, 128], fp16, tag="ones1")
        nc.vector.memset(ones1, 1.0)

        # ---- batched weight/input DMAs ------------------------------------
        # one dma_start per tensor (issue costs ~730ns of serial sync-queue
        # time each); ordered by first use: wq+xT+bq gate the first scores,
        # peT arrives in three chunks paced with the iteration-0 pe-mults.
        wq_sb = sing.tile([128, KT * E], fp16, tag="wq", name="wq")
        nc.sync.dma_start(out=wq_sb.rearrange("p (k e) -> p k e", k=KT),
                          in_=wqT_d.rearrange("(k p) e -> p k e", p=128))
        xT_sb = p_in.tile([128, KT * T], fp16, tag="xT", name="xT")
        nc.sync.dma_start(out=xT_sb.rearrange("p (k t) -> p k t", k=KT),
                          in_=xT_d.rearrange("(k p) t -> p k t", p=128))
        bq_sb = sing.tile([128, KT], f32, tag="bq", name="bq")
        nc.sync.dma_start(out=bq_sb.rearrange("p (k one) -> p k one", one=1),
                          in_=bq_d.rearrange("(k p one) -> p k one", p=128, one=1))
        ebias = sing.tile([128, 1], f32, tag="ebias")
        nc.vector.memset(ebias, EXP_SHIFT)
        peT_sb = p_in.tile([128, TT * T], fp16, tag="peT", name="peT")
        peT_dv = peT_d.rearrange("(i p) t -> p i t", p=128)
        peT_sv = peT_sb.rearrange("p (i t) -> p i t", i=TT)
        nc.sync.dma_start(out=peT_sv[:, 0:2, :], in_=peT_dv[:, 0:2, :])
        wv_sb = sing.tile([128, KT * E], fp16, tag="wv", name="wv")
        nc.sync.dma_start(out=wv_sb.rearrange("p (k e) -> p k e", k=KT),
                          in_=wvT_d.rearrange("(k p) e -> p k e", p=128))
        nc.sync.dma_start(out=peT_sv[:, 2:6, :], in_=peT_dv[:, 2:6, :])
        wo_sb = sing.tile([128, KT * E], fp16, tag="wo", name="wo")
        nc.sync.dma_start(out=wo_sb.rearrange("p (k e) -> p k e", k=KT),
                          in_=woT_d.rearrange("(k p) e -> p k e", p=128))
        nc.sync.dma_start(out=peT_sv[:, 6:8, :], in_=peT_dv[:, 6:8, :])
        bo2_sb = sing.tile([1, E], fp16, tag="bo2")
        nc.sync.dma_start(out=bo2_sb, in_=bo2_d.unsqueeze(0))

        def xT(k):
            return xT_sb[:, k * T:(k + 1) * T]

        def wq(k):
            return wq_sb[:, k * E:(k + 1) * E]

        def wv(k):
            return wv_sb[:, k * E:(k + 1) * E]

        def wo(k):
            return wo_sb[:, k * E:(k + 1) * E]

        def peT(i):
            return peT_sb[:, i * T:(i + 1) * T]

        # ---- P1: projections ----------------------------------------------
        qT_sb = [p_qv.tile([128, T], fp16, tag=f"qT{k}", name="qT") for k in range(KT)]
        v_sb = [p_qv.tile([128, H * (HD + 1)], fp16, tag=f"v{k}", name="v") for k in range(TT)]

        def emit_qproj(m, nh):
            ps = ps_b.tile([128, 512], f32, tag="slot", name="pp")
            for k in range(KT):
                nc.tensor.matmul(
                    ps, wq(k)[:, m * 128:(m + 1) * 128],
                    xT(k)[:, nh * 512:(nh + 1) * 512],
                    start=(k == 0), stop=(k == KT - 1))
            nc.vector.tensor_scalar_add(
                qT_sb[m][:, nh * 512:(nh + 1) * 512], ps, bq_sb[:, m:m + 1])

        def emit_vproj(mt):
            ps = ps_b.tile([128, 512], f32, tag="slot", name="pp")
            for k in range(KT):
                nc.tensor.matmul(
                    ps, xT(k)[:, mt * 128:(mt + 1) * 128], wv(k),
                    start=(k == 0), stop=(k == KT - 1))
            v_dst = v_sb[mt].rearrange("p (h c) -> p h c", c=HD + 1)
            nc.vector.tensor_copy(
                v_dst[:, :, 0:HD],
                ps.rearrange("p (h c) -> p h c", c=HD))
            nc.vector.memset(v_dst[:, :, HD:HD + 1], 1.0)

        for nh in range(NH):
            emit_qproj(0, nh)
        proj_rest = [("q", m, nh) for m in range(1, KT) for nh in range(NH)]
        proj_rest += [("v", mt, None) for mt in range(TT)]

        # ---- P2: attention, software-pipelined over head pairs ------------
        attnT_sb = [p_qv.tile([128, T], fp16, tag=f"attnT{k}", name="attnT") for k in range(KT)]
        Ws_of = {}   # pair j -> [hh][i] W' tiles
        at_of = {}   # pair j -> [hh][nh] psum accumulators

        def emit_scores(j, i):
            qt = qT_sb[j]
            scs = [ps_a.tile([128, T], f32, tag="slot", name="sc") for _ in range(2)]
            # hh-major: h0's psum slot is freed by exp(i-1,h0) one ACT op
            # earlier than h1's, so h0's MMs run during exp(i-1,h1) and
            # exp(i,h0) starts gap-free -- the ACT stream stays dense.
            for hh in range(2):
                r0 = hh * HD
                for nh in range(NH):
                    nc.tensor.matmul(
                        scs[hh][:, nh * 512:(nh + 1) * 512],
                        qt[r0:r0 + HD, i * 128:(i + 1) * 128],
                        qt[r0:r0 + HD, nh * 512:(nh + 1) * 512],
                        start=True, stop=True,
                        tile_position=(r0, 0))
            for hh in range(2):
                Et = p_E.tile([128, T], fp16, tag="E", name="Et")
                nc.scalar.activation(out=Et, in_=scs[hh], func=Exp, scale=0.125, bias=ebias)
                Wt = p_W.tile([128, T], fp16, tag="W", name="Wt")
                if hh == 0 and (j == 0 or i in (4, 5, 6)):
                    nc.gpsimd.tensor_mul(Wt, Et, peT(i))
                else:
                    nc.vector.tensor_mul(Wt, Et, peT(i))
                Ws_of[j][hh][i] = Wt

        def alloc_at(j, pools=None):
            # [hh][nh] accumulators; pools[nh] selects the psum pool per nh
            pools = pools or [ps_b, ps_b]
            at_of[j] = [[pools[nh].tile([HD + 1, 512], f32, tag="slot", name="at")
                         for nh in range(NH)] for _ in range(2)]

        def emit_attn_kstep(j, i):
            for hh in range(2):
                vcol = 65 * (2 * j + hh)
                for nh in range(NH):
                    nc.tensor.matmul(
                        at_of[j][hh][nh],
                        v_sb[i][:, vcol:vcol + HD + 1],
                        Ws_of[j][hh][i][:, nh * 512:(nh + 1) * 512],
                        start=(i == 0), stop=(i == TT - 1))

        rq_of = {}

        def emit_chain_a(j, nh):
            # normalization, part a (per t-half): the two denominator rows ->
            # one (1,1024) SBUF row, DMA-reshaped to (16,64) so ONE exact
            # reciprocal covers them at 64 elems/lane (recip is ~6 cyc/elem
            # and free-dim bound), DMA'd back row-shaped for the broadcast
            # matmuls.  SBUF->SBUF DMAs only, no DRAM bounce.
            rr = p_rr.tile([HD + 1, 2 * 512], f32, tag=f"rr{nh}", name="rr")
            for hh in range(2):
                nc.vector.tensor_copy(
                    rr[HD:HD + 1, hh * 512:(hh + 1) * 512],
                    at_of[j][hh][nh][HD:HD + 1, :])
            rg = p_rr.tile([16, 64], f32, tag=f"rg{nh}", name="rg")
            nc.sync.dma_start(
                out=rg,
                in_=rr[HD:HD + 1, :].rearrange("one (a c) -> one a c", c=64))
            rgi = p_rr.tile([16, 64], f32, tag=f"rgi{nh}", name="rgi")
            nc.vector.reciprocal(rgi, rg)
            rq = p_rr.tile([HD + 1, 2 * 512], f32, tag=f"rq{nh}", name="rq")
            nc.sync.dma_start(
                out=rq[HD:HD + 1, :].rearrange("one (a c) -> one a c", c=64),
                in_=rgi)
            rq_of[(j, nh)] = rq

        def emit_chain_b(j, nh, bc_pool):
            # part b: two accumulated K=1 selector matmuls broadcast both
            # head-halves' reciprocal rows into one (128,512) psum (PE), one
            # psum->sbuf copy, fused evacuation muls (attnT = psum * rm).
            rq = rq_of.pop((j, nh))
            rp = bc_pool.tile([128, 512], f32, tag="slot", name="rp")
            for hh in range(2):
                nc.tensor.matmul(
                    rp, sel_h[hh][HD:HD + 1, :],
                    rq[HD:HD + 1, hh * 512:(hh + 1) * 512],
                    start=(hh == 0), stop=(hh == 1),
                    tile_position=(HD, 0))
            rm = p_rm.tile([128, 512], f32, tag=f"rm{nh}", name="rm")
            nc.vector.tensor_copy(rm, rp)
            for hh in range(2):
                nc.vector.tensor_mul(
                    attnT_sb[j][hh * HD:(hh + 1) * HD,
                                nh * 512:(nh + 1) * 512],
                    at_of[j][hh][nh][0:HD, :],
                    rm[hh * HD:(hh + 1) * HD, :])

        # ---- main pipelined loop ------------------------------------------
        for j in range(NP):
            Ws_of[j] = [[None] * TT, [None] * TT]
            if j - 2 in Ws_of:
                del Ws_of[j - 2]
            attn_q = []
            if j >= 1:
                alloc_at(j - 1)
                attn_q = list(range(TT))
            for i in range(TT):
                emit_scores(j, i)
                # pair j-2's chain part b lands at slots 2-3: the reshape
                # DMAs (issued at iteration j-1's end) are long done, so
                # neither the bcMM nor the rm-copy ever head-block a queue
                if j >= 2 and i in (2, 3):
                    emit_chain_b(j - 2, i - 2, ps_a)
                if j == 0:
                    for _ in range(2):
                        if proj_rest:
                            kind, a1, a2 = proj_rest.pop(0)
                            if kind == "q":
                                emit_qproj(a1, a2)
                            else:
                                emit_vproj(a1)
                else:
                    start = 3 if j >= 2 else 1
                    if i >= start:
                        take = 2 if i >= start + 2 and attn_q and len(attn_q) > (TT - 1 - i) else 1
                        for _ in range(take):
                            if attn_q:
                                emit_attn_kstep(j - 1, attn_q.pop(0))
            for i in attn_q:
                emit_attn_kstep(j - 1, i)
            if j >= 1:
                for nh in range(NH):
                    emit_chain_a(j - 1, nh)

        # ---- tail ----------------------------------------------------------
        p3_part = [p_qv.tile([128, E], f32, tag=f"p3p{k}", name="p3p")
                   for k in range(TT)]

        def emit_p3a(mt):
            ps = ps_a.tile([128, 512], f32, tag="slot", name="pp")
            for k in range(KT - 1):
                nc.tensor.matmul(
                    ps, attnT_sb[k][:, mt * 128:(mt + 1) * 128],
                    wo(k), start=(k == 0), stop=(k == KT - 2))
            nc.vector.tensor_copy(p3_part[mt], ps)

        # finish pair 2's chains (bc tiles on ps_a: its ring holds the last
        # scores slots, already exp'd).  Two p3a partials (they only need
        # attnT(0..2)) keep the PE fed, then pair 3's attention nh-major:
        # nh0's accumulators on ps_a (aliasing the p3a tiles), nh1's on
        # ps_b (aliasing at(2), freed by pair-2's muls).  Each nh's chain
        # part a runs while the other nh's matmuls (or the p3a partials)
        # keep the PE busy, so the reshape-DMA latency never idles the PE;
        # pair-3's broadcast tiles go to ps_b.
        def emit_attn_knh(j, i, nh):
            for hh in range(2):
                vcol = 65 * (2 * j + hh)
                nc.tensor.matmul(
                    at_of[j][hh][nh],
                    v_sb[i][:, vcol:vcol + HD + 1],
                    Ws_of[j][hh][i][:, nh * 512:(nh + 1) * 512],
                    start=(i == 0), stop=(i == TT - 1))

        emit_chain_b(NP - 2, 0, ps_a)
        emit_chain_b(NP - 2, 1, ps_a)
        emit_p3a(0)
        emit_p3a(1)
        alloc_at(NP - 1, pools=[ps_a, ps_b])
        for nh in range(NH):
            for i in range(TT):
                emit_attn_knh(NP - 1, i, nh)
            emit_chain_a(NP - 1, nh)
        emit_chain_b(NP - 1, 0, ps_b)
        for mt in range(2, TT):
            emit_p3a(mt)
        emit_chain_b(NP - 1, 1, ps_b)

        # k=3 + bo2 bias, add the partial, store (2 mt per DMA)
        st_tiles = {}
        for mt in range(TT):
            ps = ps_a.tile([128, 512], f32, tag="slot", name="pp")
            nc.tensor.matmul(
                ps, attnT_sb[KT - 1][:, mt * 128:(mt + 1) * 128],
                wo(KT - 1), start=True, stop=False)
            nc.tensor.matmul(ps, ones1, bo2_sb, start=False, stop=True)
            g, h = divmod(mt, 2)
            if h == 0:
                st_tiles[g] = p_st.tile([128, 2 * E], f32, tag="st", name="st")
            st = st_tiles[g]
            nc.vector.tensor_add(st[:, h * E:(h + 1) * E], ps, p3_part[mt])
            if h == 1:
                nc.sync.dma_start(
                    out=out_d[g * 256:(g + 1) * 256, :].rearrange(
                        "(two p) e -> p two e", p=128),
                    in_=st.rearrange("p (two e) -> p two e", two=2))

    nc.compile()
    return nc


def get_nc():
    if "nc" not in _cache:
        _cache["nc"] = _build_nc()
    return _cache["nc"]


def prep_inputs(query, pe, in_proj_weight, in_proj_bias, out_proj_weight,
                out_proj_bias):
    """Host-side sharding/layout prep. Returns per-core input maps."""
    query = np.asarray(query, dtype=np.float32)
    pe = np.asarray(pe, dtype=np.float32)
    in_proj_weight = np.asarray(in_proj_weight, dtype=np.float32)
    in_proj_bias = np.asarray(in_proj_bias, dtype=np.float32)
    out_proj_weight = np.asarray(out_proj_weight, dtype=np.float32)
    out_proj_bias = np.asarray(out_proj_bias, dtype=np.float32)

    wqT = np.ascontiguousarray(in_proj_weight[0:E].T).astype(np.float16)
    wvT = np.ascontiguousarray(in_proj_weight[2 * E:3 * E].T).astype(np.float16)
    woT = np.ascontiguousarray(out_proj_weight.T).astype(np.float16)
    bq = np.ascontiguousarray(in_proj_bias[0:E])
    bv = in_proj_bias[2 * E:3 * E]
    bo2 = (out_proj_weight @ bv + out_proj_bias).astype(np.float16)

    in_maps = []
    for b in range(N_CORES):
        xT = np.ascontiguousarray(query[:, b, :].T).astype(np.float16)
        peT = np.ascontiguousarray(pe[b].T).astype(np.float16)
        in_maps.append({
            "xT": xT, "peT": peT, "wqT": wqT, "wvT": wvT, "woT": woT,
            "bq": bq, "bo2": bo2,
        })
    return in_maps


def kernel(query, pe, in_proj_weight, in_proj_bias, out_proj_weight,
           out_proj_bias):
    from concourse.bass_utils import run_bass_kernel_spmd

    nc = get_nc()
    in_maps = prep_inputs(query, pe, in_proj_weight, in_proj_bias,
                          out_proj_weight, out_proj_bias)
    res = run_bass_kernel_spmd(nc, in_maps, list(range(N_CORES)))
    out = np.empty((T, B, E), dtype=np.float32)
    for b in range(N_CORES):
        out[:, b, :] = res.results[b]["out"]
    return out
